# revision 1
# baseline (speedup 1.0000x reference)
"""Causal self-attention (B=2, T=2048, C=1024, H=16) on 8 TRN2 NeuronCores.

Sharding: core c -> batch b = c // 4, head group = heads [4*(c%4) .. 4*(c%4)+4).
Each core computes qkv for its 4 heads on its batch, causal attention, and a
row-parallel partial of the output projection (over its 256 head channels).
The host sums the 4 partials per batch; b_proj/4 is added on-device so the sum
reproduces a single b_proj add.

All device tensors are pre-transposed on the host so the kernel never
transposes on-chip:
  xt   [C, T]    = x[b].T                     (bf16)
  wqkt [C, 512]  = w_attn[qk rows].T          (bf16)  cols: q_h0 q_h1 q_h2 q_h3 k_h0..k_h3
  wvt  [C, 256]  = w_attn[v rows].T           (bf16)
  wpt  [256, C]  = w_proj[:, head cols].T     (bf16)
  out_t[C, T]    = partial (x @ w_proj.T).T   (fp32)

On-chip dataflow (per head pair, row/col layouts chosen so the TensorE
contraction dim is always the partition dim and no transposes are needed):
  qT,kT [d, t] -> S^T[tk, tq] (two heads packed in the 128-wide array via
  row tiling) -> exp on ScalarE (scale=1/8 folded in) -> causal mask via
  static 0/1 mask multiply on DVE -> AV matmul with V augmented by a ones
  column (denominator accumulates in row 64 of PSUM for free) -> reciprocal +
  K=2 broadcast matmul to spread 1/den across 64 partitions -> normalize ->
  projection (stays transposed).
"""

import os
import sys
import types

import numpy as np
import ml_dtypes

import concourse.bass as bass
import concourse.mybir as mybir
import concourse.tile as tile
from concourse import bacc
from concourse.hw_specs import get_activation_tables

BF16 = ml_dtypes.bfloat16


class _Bacc(bacc.Bacc):
    """Bacc that steers Exp/Ln activations to the combined
    natural_log_exp_and_others table set so the kernel never swaps
    activation tables (set ids keep their act_info.json positions)."""

    def insert_act_table_loads(self):
        import bass_rust as _br
        import concourse.mybir as _mybir

        has_activation = any(
            isinstance(i, _mybir.InstActivation)
            for b in self.main_func.blocks
            for i in b.instructions
        )
        if not has_activation:
            return
        combined = {"natural_log_exp_and_others"}
        steer = {_mybir.ActivationFunctionType.Exp, _mybir.ActivationFunctionType.Ln}
        tables = []
        for name, fns in get_activation_tables(self.m.arch).items():
            if name not in combined:
                fns = {f for f in fns if f not in steer}
            tables.append((name, set(fns)))
        _br.insert_act_table_loads(self, tables)

B, T, C = 2, 2048, 1024
H = 16
DH = 64
N_CORES = 8
HEADS_PER_CORE = 4
TQ = 512          # tq tile (moving dim of scores/AV matmuls)
TK = 128          # tk tile (PSUM partition dim of S^T)
NG = T // TQ      # 4 tq tiles
NKT = T // TK     # 16 tk tiles
NC_ = C // 128    # 8 contraction tiles for the qkv matmuls
FP32 = mybir.dt.float32
BF16_DT = mybir.dt.bfloat16
import os as _os
DEPTH = int(_os.environ.get("K_DEPTH", "4"))
POPS_EVERY = int(_os.environ.get("K_POPS_EVERY", "2"))
BOUNDARY_POPS = int(_os.environ.get("K_BPOPS", "3"))
WARMUP_MMS = int(_os.environ.get("K_WARMUP", "24"))
PT_BUFS = int(_os.environ.get("K_PT_BUFS", "6"))


def _ensure_axon_hooks_stub():
    """bass_utils imports antenv.axon_hooks when trace is requested (even via
    the BASS_TRACE env var). The container's antenv stub lacks that module, so
    install a minimal one to keep the no-trace fallback path working."""
    try:
        import antenv  # noqa: F401
    except ImportError:
        return
    if "antenv.axon_hooks" in sys.modules:
        return
    try:
        import antenv.axon_hooks  # noqa: F401
        return
    except ImportError:
        pass
    mod = types.ModuleType("antenv.axon_hooks")
    mod._hook = None

    def set_axon_ntff_profile_hook(h):
        mod._hook = h

    def get_axon_ntff_profile_hook():
        return mod._hook

    mod.set_axon_ntff_profile_hook = set_axon_ntff_profile_hook
    mod.get_axon_ntff_profile_hook = get_axon_ntff_profile_hook
    sys.modules["antenv.axon_hooks"] = mod
    import antenv as _a

    _a.axon_hooks = mod


def build_bass():
    """Emit the single-core SPMD Bass module (same program on all 8 cores).

    Round 2: software-pipelined attention (PE never waits on exp), pair-wide
    2-bank PSUM score tiles with a single exp instruction per (pair, kt),
    causal column truncation on diagonal tiles, and qkv/proj matmul groups
    interleaved into the attention stream as filler PE work.
    """
    from collections import deque
    from contextlib import ExitStack

    nc = _Bacc("TRN2", target_bir_lowering=False, debug=False)

    xt = nc.declare_dram_parameter("xt", [C, T], BF16_DT, isOutput=False).ap()
    wqkt = nc.declare_dram_parameter("wqkt", [C, 512], BF16_DT, isOutput=False).ap()
    wvt = nc.declare_dram_parameter("wvt", [C, 256], BF16_DT, isOutput=False).ap()
    wpt = nc.declare_dram_parameter("wpt", [256, C], BF16_DT, isOutput=False).ap()
    bqk = nc.declare_dram_parameter("bqk", [512, 1], FP32, isOutput=False).ap()
    bv = nc.declare_dram_parameter("bv", [128, 256], FP32, isOutput=False).ap()
    bp = nc.declare_dram_parameter("bp", [C, 1], FP32, isOutput=False).ap()
    out_t = nc.declare_dram_parameter("out_t", [C, T], FP32, isOutput=True).ap()

    Exp = mybir.ActivationFunctionType.Exp
    mult = mybir.AluOpType.mult
    add = mybir.AluOpType.add
    is_ge = mybir.AluOpType.is_ge

    with tile.TileContext(nc) as tc, ExitStack() as ctx:
        res = ctx.enter_context(tc.tile_pool(name="resident", bufs=1))

        # --- resident loads -------------------------------------------------
        # Load exactly what unit (0,0) and the first qkv groups need first
        # (xt g0-slices, wqk q01/k01 columns, wv), split across both HWDGE
        # queues; the bulk of xt and the remaining wqk columns follow.
        xt_t = [res.tile([128, T], BF16_DT, tag=f"xt{i}", name=f"xt{i}")
                for i in range(NC_)]
        wqk_t = [res.tile([128, 512], BF16_DT, tag=f"wqk{i}", name=f"wqk{i}")
                 for i in range(NC_)]
        wv_t = [res.tile([128, 256], BF16_DT, tag=f"wv{i}", name=f"wv{i}")
                for i in range(NC_)]
        bqk_bt = res.tile([128, 4], FP32, tag="bqkb", name="bqk_bt")
        bqk_t = [bqk_bt[:, j : j + 1] for j in range(4)]
        bp_bt = res.tile([128, 8], FP32, tag="bpb", name="bp_bt")
        bp_t = [bp_bt[:, j : j + 1] for j in range(8)]
        wp_t = [res.tile([128, C], BF16_DT, tag=f"wp{i}", name=f"wp{i}")
                for i in range(2)]
        bv_t = res.tile([128, 256], FP32, tag="bv", name="bv")

        # Priority-ordered input loads, round-robined over both HWDGE queues
        # (each queue is a serial chain of ~1.5us/transfer): criticals for
        # unit (0,0) first.  wqk is split into the q01/k01 column pairs
        # (needed first) and the q23/k23 pairs via strided APs.
        def _wqk_half(i, off):
            r = slice(128 * i, 128 * (i + 1))
            sb = wqk_t[i][:].rearrange("p (a c) -> p a c", c=128)[:, off::2, :]
            dr = wqkt[r, :].rearrange("p (a c) -> p a c", c=128)[:, off::2, :]
            return sb, dr

        prio = []
        for i in range(NC_):
            prio.append((xt_t[i][:, 0:512], xt[128 * i : 128 * (i + 1), 0:512]))
        for i in range(NC_):
            prio.append(_wqk_half(i, 0))
        prio.append((bqk_bt[:], bqk.rearrange("(j p) o -> p (j o)", p=128)))
        for i in range(NC_):
            prio.append((wv_t[i][:], wvt[128 * i : 128 * (i + 1), :]))
            prio.append(_wqk_half(i, 1))
        prio.append((bp_bt[:], bp.rearrange("(j p) o -> p (j o)", p=128)))
        prio.append((bv_t[:], bv[:]))
        for n, (dst, src_ap) in enumerate(prio):
            (nc.sync if n % 2 == 0 else nc.scalar).dma_start(dst, src_ap)

        # Single causal strip mask [128, 128]: keep iff local tq >= local tk.
        maskd = res.tile([128, 128], BF16_DT, tag="maskd", name="maskd")
        nc.gpsimd.memset(maskd[:], 1.0)
        nc.gpsimd.affine_select(
            out=maskd[:], in_=maskd[:], compare_op=is_ge, fill=0.0,
            base=0, pattern=[[1, 128]], channel_multiplier=-1,
        )

        # Ones row (lane 64, matching the av_* denominator row) for the K=1
        # broadcast matmuls.
        ones_t = res.tile([65, 64], BF16_DT, tag="ones_t", name="ones_t")
        nc.vector.memset(ones_t[:], 1.0)

        # qT/kT in [head-channel, t] layout: tile p holds heads (2p, 2p+1).
        qk_sb = [
            res.tile([128, T], BF16_DT, tag=f"qk{i}", name=f"qk{i}") for i in range(4)
        ]
        # V natural [t, d] with a ones column after each head: 4*(64+1) cols.
        v_sb = []
        for i in range(NKT):
            t = res.tile([128, 260], BF16_DT, tag=f"v{i}", name=f"v{i}")
            nc.gpsimd.memset(
                t[:].rearrange("p (h c) -> p h c", c=65)[:, :, 64:65], 1.0
            )
            v_sb.append(t)
        att_sb = [
            res.tile([128, T], BF16_DT, tag=f"att{i}", name=f"att{i}")
            for i in range(2)
        ]

        # bulk of xt and the projection weights arrive via the (otherwise
        # idle at startup) SWDGE path, after the gpsimd mask/ones builders
        for i in range(NC_):
            nc.gpsimd.dma_start(
                xt_t[i][:, 512:T], xt[128 * i : 128 * (i + 1), 512:T]
            )
        for i in range(2):
            nc.gpsimd.dma_start(wp_t[i][:], wpt[128 * i : 128 * (i + 1), :])

        sc_ps = ctx.enter_context(tc.tile_pool(name="sc_ps", bufs=2, space="PSUM"))
        av_ps = ctx.enter_context(tc.tile_pool(name="av_ps", bufs=2, space="PSUM"))
        qv_ps = ctx.enter_context(tc.tile_pool(name="qv_ps", bufs=1, space="PSUM"))
        bp_ps = ctx.enter_context(tc.tile_pool(name="bp_ps", bufs=1, space="PSUM"))
        pt_pool = ctx.enter_context(tc.tile_pool(name="pt_pool", bufs=PT_BUFS))
        riv_pool = ctx.enter_context(tc.tile_pool(name="riv", bufs=2))
        bcs_pool = ctx.enter_context(tc.tile_pool(name="bcs", bufs=2))
        scr_pool = ctx.enter_context(tc.tile_pool(name="scr", bufs=2))
        osb_pool = ctx.enter_context(tc.tile_pool(name="osb", bufs=4))

        # --- filler work: qkv projections + output projection --------------
        emitted = set()

        def emit_qk_group(jt, g):
            ps = qv_ps.tile([128, 512], FP32, tag="qv", name=f"qkps{jt}_{g}")
            for ct in range(NC_):
                nc.tensor.matmul(
                    ps[:],
                    lhsT=wqk_t[ct][:, 128 * jt : 128 * (jt + 1)],
                    rhs=xt_t[ct][:, TQ * g : TQ * (g + 1)],
                    start=(ct == 0),
                    stop=(ct == NC_ - 1),
                )

            nc.vector.tensor_scalar(
                qk_sb[jt][:, TQ * g : TQ * (g + 1)], ps[:], bqk_t[jt][:], None,
                op0=add,
            )

        def emit_v_group(tt):
            ps = qv_ps.tile([128, 512], FP32, tag="qv", name=f"vps{tt}")
            for ct in range(NC_):
                nc.tensor.matmul(
                    ps[:, 0:256],
                    lhsT=xt_t[ct][:, 128 * tt : 128 * (tt + 1)],
                    rhs=wv_t[ct][:],
                    start=(ct == 0),
                    stop=(ct == NC_ - 1),
                )

            vt = v_sb[tt]
            nc.vector.tensor_tensor(
                out=vt[:].rearrange("p (h c) -> p h c", c=65)[:, :, 0:64],
                in0=ps[:, 0:256].rearrange("p (h c) -> p h c", c=64),
                in1=bv_t[:].rearrange("p (h c) -> p h c", c=64),
                op=add,
            )

        def emit_proj_group(jt, g, pool=None, tag="bp"):
            tqs = slice(TQ * g, TQ * (g + 1))
            pp = (pool or bp_ps).tile([128, 512], FP32, tag=tag, name=f"pj{g}{jt}")
            nc.tensor.matmul(
                pp[:], lhsT=wp_t[0][:, 128 * jt : 128 * (jt + 1)],
                rhs=att_sb[0][:, tqs], start=True, stop=False,
            )
            nc.tensor.matmul(
                pp[:], lhsT=wp_t[1][:, 128 * jt : 128 * (jt + 1)],
                rhs=att_sb[1][:, tqs], start=False, stop=True,
            )
            osb = osb_pool.tile([128, 512], FP32, tag="osb", name=f"osb{g}{jt}")
            nc.vector.tensor_scalar(osb[:], pp[:], bp_t[jt][:], None, op0=add)
            nc.sync.dma_start(out_t[128 * jt : 128 * (jt + 1), tqs], osb[:])

        work_q = deque()

        # --- PE warm-up: ~5us of dense zero matmuls while the DMAs stream in,
        # so the HAM clock gate opens before real compute starts ------------
        warm_sb = res.tile([128, 512], BF16_DT, tag="warm", name="warm_sb")
        nc.vector.memset(warm_sb[:], 0.0)
        warm_ps = qv_ps.tile([128, 512], FP32, tag="qv", name="warm_ps")
        for i in range(WARMUP_MMS):
            nc.tensor.matmul(
                warm_ps[:], lhsT=warm_sb[:, 0:128], rhs=warm_sb[:],
                start=(i == 0), stop=(i == WARMUP_MMS - 1), skip_group_check=True,
            )

        # Dummy zero-matmuls to keep the PE clock gate open when real filler
        # runs dry (late units and the projection tail).
        hb_n = [0]

        def heartbeat(n=2, pool=None, tag="qv"):
            t = (pool or qv_ps).tile([128, 512], FP32, tag=tag,
                                     name=f"hb{hb_n[0]}")
            hb_n[0] += 1
            for i in range(n):
                nc.tensor.matmul(
                    t[:], lhsT=warm_sb[:, 0:128], rhs=warm_sb[:],
                    start=(i == 0), stop=(i == n - 1), skip_group_check=True,
                )

        def emit_item(item):
            if item[0] == "qk":
                emit_qk_group(item[1], item[2])
            elif item[0] == "v":
                emit_v_group(item[1])
            else:
                emit_proj_group(item[1], item[2])
            emitted.add(item)

        def pop_one(force=False):
            if work_q:
                emit_item(work_q.popleft())

        def drain_until(needed):
            for item in needed:
                while item not in emitted:
                    emit_item(work_q.popleft())

        # prologue: enough qkv for unit (0, 0), rest queued in dep-safe order
        for item in [("qk", 0, 0), ("qk", 2, 0), ("v", 0), ("v", 1), ("v", 2),
                     ("v", 3)]:
            emit_item(item)
        work_q.extend([("qk", 1, 0), ("qk", 3, 0)])
        for gg in range(1, NG):
            work_q.extend(
                [("qk", 2, gg), ("qk", 0, gg), ("qk", 3, gg), ("qk", 1, gg)]
                + [("v", 4 * gg + i) for i in range(4)]
            )

        # --- attention: software-pipelined units -----------------------------
        def norm_pre(g, p, av_e, av_o):
            """1/den via exp(-ln(den)) on ScalarE (Ln and Exp share one
            activation table set, so no table swaps)."""
            Ln = mybir.ActivationFunctionType.Ln
            lr_e = riv_pool.tile([65, 512], FP32, tag="lr", name=f"lre{g}{p}")
            lr_o = riv_pool.tile([65, 512], FP32, tag="lr", name=f"lro{g}{p}")
            nc.scalar.activation(lr_e[64:65, :], av_e[64:65, :], Ln)
            nc.scalar.activation(lr_o[64:65, :], av_o[64:65, :], Ln)
            rb_e = riv_pool.tile([65, 512], BF16_DT, tag="rb", name=f"rbe{g}{p}")
            rb_o = riv_pool.tile([65, 512], BF16_DT, tag="rb", name=f"rbo{g}{p}")
            nc.scalar.activation(rb_e[64:65, :], lr_e[64:65, :], Exp, scale=-1.0)
            nc.scalar.activation(rb_o[64:65, :], lr_o[64:65, :], Exp, scale=-1.0)
            return rb_e, rb_o

        def norm_post(g, p, av_e, av_o, riv_e, riv_o):
            """Broadcast 1/den across 64 partitions (K=1 fp32r matmul) and
            normalize; enqueues proj work for p==1."""
            tqs = slice(TQ * g, TQ * (g + 1))
            bc_e = bp_ps.tile([64, 512], FP32, tag="bp", name=f"bce{g}{p}")
            nc.tensor.matmul(
                bc_e[:], lhsT=ones_t[64:65, :], rhs=riv_e[64:65, :],
                start=True, stop=True, tile_position=(64, 0),
            )
            bcs_e = bcs_pool.tile([64, 512], FP32, tag="bcs", name=f"bcse{g}{p}")
            nc.vector.tensor_copy(out=bcs_e[:], in_=bc_e[:])
            nc.vector.tensor_tensor(
                out=att_sb[p][0:64, tqs], in0=av_e[0:64, :], in1=bcs_e[:], op=mult
            )
            bc_o = bp_ps.tile([64, 512], FP32, tag="bp", name=f"bco{g}{p}")
            nc.tensor.matmul(
                bc_o[:], lhsT=ones_t[64:65, :], rhs=riv_o[64:65, :],
                start=True, stop=True, tile_position=(64, 0),
            )
            bcs_o = bcs_pool.tile([64, 512], FP32, tag="bcs", name=f"bcso{g}{p}")
            nc.vector.tensor_copy(out=bcs_o[:], in_=bc_o[:])
            scr = scr_pool.tile([64, 512], BF16_DT, tag="scr", name=f"scr{g}{p}")
            nc.vector.tensor_tensor(
                out=scr[:], in0=av_o[0:64, :], in1=bcs_o[:], op=mult
            )
            nc.sync.dma_start(att_sb[p][64:128, tqs], scr[:])
            if p == 1:
                work_q.extend([("proj", jt, g) for jt in range(8)])

        pending_norm = None
        for g, p in [(0, 0), (0, 1), (1, 0), (1, 1), (2, 0), (2, 1),
                     (3, 0), (3, 1)]:
                nkt = 4 * (g + 1)
                h_e, h_o = 2 * p, 2 * p + 1
                q_t, k_t = qk_sb[p], qk_sb[2 + p]
                tq0 = TQ * g
                drain_until(
                    [("qk", p, g)]
                    + [("qk", 2 + p, gg) for gg in range(g + 1)]
                    + [("v", t) for t in range(nkt)]
                )
                s_tiles = {}
                p_tiles = {}
                av_e = av_o = None

                def lo_of(kt, g=g):
                    i = kt - 4 * g
                    return 128 * i if i > 0 else 0

                def scores(kt, g=g, q_t=q_t, k_t=k_t, tq0=tq0, p=p):
                    lo = lo_of(kt, g)
                    s_pair = sc_ps.tile([128, 1024], FP32, tag="sc",
                                        name=f"s{g}{p}{kt}")
                    kts = slice(128 * kt, 128 * (kt + 1))
                    rq = slice(tq0 + lo, tq0 + 512)
                    nc.tensor.matmul(
                        s_pair[:, lo:512], lhsT=k_t[0:64, kts], rhs=q_t[0:64, rq],
                        start=True, stop=True,
                    )
                    nc.tensor.matmul(
                        s_pair[:, 512 + lo : 1024], lhsT=k_t[64:128, kts],
                        rhs=q_t[64:128, rq], start=True, stop=True,
                        tile_position=(64, 0),
                    )
                    s_tiles[kt] = s_pair

                def expmask(kt, g=g, p=p):
                    lo = lo_of(kt, g)
                    s_pair = s_tiles.pop(kt)
                    p_pair = pt_pool.tile([128, 1024], BF16_DT, tag="pt",
                                          name=f"p{g}{p}{kt}")
                    s3 = s_pair[:].rearrange("p (h c) -> p h c", c=512)[:, :, lo:512]
                    p3 = p_pair[:].rearrange("p (h c) -> p h c", c=512)[:, :, lo:512]
                    nc.scalar.activation(p3, s3, Exp, scale=0.125)
                    if kt >= 4 * g:  # diagonal: mask the leading 128-wide strip
                        pm = p_pair[:].rearrange("p (h c) -> p h c", c=512)[
                            :, :, lo : lo + 128
                        ]
                        mk = maskd[:, None, 0:128].to_broadcast([128, 2, 128])
                        nc.gpsimd.tensor_tensor(out=pm, in0=pm, in1=mk, op=mult)
                    p_tiles[kt] = p_pair

                def av_mm(kt, g=g, p=p, nkt=nkt, h_e=h_e, h_o=h_o):
                    lo = lo_of(kt, g)
                    p_pair = p_tiles.pop(kt)
                    nc.tensor.matmul(
                        av_e[:, lo:512], lhsT=v_sb[kt][:, 65 * h_e : 65 * h_e + 65],
                        rhs=p_pair[:, lo:512], start=(kt == 0),
                        stop=(kt == nkt - 1), skip_group_check=True,
                    )
                    nc.tensor.matmul(
                        av_o[:, lo:512], lhsT=v_sb[kt][:, 65 * h_o : 65 * h_o + 65],
                        rhs=p_pair[:, 512 + lo : 1024], start=(kt == 0),
                        stop=(kt == nkt - 1), skip_group_check=True,
                    )

                depth = min(DEPTH, nkt)
                for kt in range(2):
                    scores(kt)
                for kt in range(2):
                    expmask(kt)
                for kt in range(2, depth):
                    scores(kt)
                    expmask(kt)
                if pending_norm is not None:
                    rivs = norm_pre(*pending_norm)
                    for _ in range(BOUNDARY_POPS):
                        pop_one()
                    norm_post(*pending_norm, *rivs)
                av_e = av_ps.tile([65, 512], FP32, tag="av", name=f"ave{g}{p}")
                av_o = av_ps.tile([65, 512], FP32, tag="av", name=f"avo{g}{p}")
                for kt in range(depth, nkt):
                    scores(kt)
                    expmask(kt)
                    if kt % POPS_EVERY == 0:
                        if work_q:
                            pop_one()
                        elif g >= 2:
                            heartbeat(3)
                    av_mm(kt - depth)
                for kt in range(nkt - depth, nkt):
                    av_mm(kt)
                pending_norm = (g, p, av_e, av_o)

        # epilogue: last unit's normalize + remaining proj groups (these can
        # rotate through the now-idle 2-bank score slots for more overlap)
        rivs = norm_pre(*pending_norm)
        norm_post(*pending_norm, *rivs)
        while work_q:
            item = work_q.popleft()
            if item[0] == "proj":
                emit_proj_group(item[1], item[2], pool=sc_ps, tag="sc")
                emitted.add(item)
                heartbeat(4)
            else:
                emit_item(item)

    nc.compile()
    return nc


_NC_CACHE = None


def _get_nc():
    global _NC_CACHE
    if _NC_CACHE is None:
        _NC_CACHE = build_bass()
    return _NC_CACHE


def make_in_maps(x, w_attn, b_attn, w_proj, b_proj):
    """Host-side sharding: slice/transpose/cast the full inputs per core."""
    x = np.asarray(x, dtype=np.float32)
    w_attn = np.asarray(w_attn, dtype=np.float32)
    b_attn = np.asarray(b_attn, dtype=np.float32)
    w_proj = np.asarray(w_proj, dtype=np.float32)
    b_proj = np.asarray(b_proj, dtype=np.float32)
    in_maps = []
    for core in range(N_CORES):
        b = core // 4
        heads = [4 * (core % 4) + i for i in range(HEADS_PER_CORE)]
        ch = np.concatenate([np.arange(h * DH, (h + 1) * DH) for h in heads])
        idx_qk = np.concatenate([ch, C + ch])
        idx_v = 2 * C + ch
        in_maps.append(
            {
                "xt": np.ascontiguousarray(x[b].T).astype(BF16),
                "wqkt": np.ascontiguousarray(w_attn[idx_qk].T).astype(BF16),
                "wvt": np.ascontiguousarray(w_attn[idx_v].T).astype(BF16),
                "wpt": np.ascontiguousarray(w_proj[:, ch].T).astype(BF16),
                "bqk": b_attn[idx_qk].astype(np.float32).reshape(512, 1),
                "bv": np.tile(b_attn[idx_v].astype(np.float32)[None, :], (128, 1)),
                "bp": (b_proj / 4.0).astype(np.float32).reshape(C, 1),
            }
        )
    return in_maps


def assemble_output(results):
    out = np.zeros((B, T, C), dtype=np.float32)
    for core in range(N_CORES):
        out[core // 4] += np.asarray(results[core]["out_t"], dtype=np.float32).T
    return out


def run(inputs, trace=False, trace_cores=None, tmpdir=None):
    """Run on hardware; returns (output, BassKernelResults)."""
    _ensure_axon_hooks_stub()
    from concourse.bass_utils import run_bass_kernel_spmd

    nc = _get_nc()
    in_maps = make_in_maps(**inputs)
    kw = {}
    if trace:
        kw.update(trace=True, trace_cores=trace_cores, tmpdir=tmpdir)
    res = run_bass_kernel_spmd(nc, in_maps, core_ids=list(range(N_CORES)), **kw)
    return assemble_output(res.results), res


def kernel(x, w_attn, b_attn, w_proj, b_proj):
    out, _ = run(
        dict(x=x, w_attn=w_attn, b_attn=b_attn, w_proj=w_proj, b_proj=b_proj)
    )
    return out



# revision 5
# speedup vs baseline: 1.1237x; 1.1237x over previous
"""Causal self-attention (B=2, T=2048, C=1024, H=16) on 8 TRN2 NeuronCores.

Sharding: core c -> batch b = c // 4, head group = heads [4*(c%4) .. 4*(c%4)+4).
Each core computes qkv for its 4 heads on its batch, causal attention, and a
row-parallel partial of the output projection (over its 256 head channels).
The host sums the 4 partials per batch; b_proj/4 is added on-device so the sum
reproduces a single b_proj add.  Output partials stream back in bf16.

All device tensors are pre-transposed on the host so the kernel never
transposes on-chip:
  xt   [C, T]    = x[b].T                     (bf16)
  wqkt [C, 512]  = w_attn[qk rows].T          (bf16)  cols: q_h0 q_h1 q_h2 q_h3 k_h0..k_h3
  wvt  [C, 256]  = w_attn[v rows].T           (bf16)
  wpt  [256, C]  = w_proj[:, head cols].T     (bf16)
  bias [128,268] = packed bqk(4) | bp(8) | bv(256) columns (fp32)
  out_t[C, T]    = partial (x @ w_proj.T).T   (bf16)

Round 3 (vs the round-2 baseline at ~194us):
  * input DMAs coalesced into 7 big multi-dim-AP transfers (xt in four
    1MB t-blocks on sync, wqk/wv on scalar, bias/wp on gpsimd) so the
    first qk group can start ~6us in instead of ~22us;
  * softmax 1/den moved off ScalarE: DVE copies the PSUM den row to bf16,
    a K=1 matmul broadcasts den across 64 partitions, and
    reciprocal_approx_fast (custom DVE op, ~18 bits) inverts it -- the
    Ln/Exp activation chain is gone (ScalarE now runs score exps only);
  * av pair kept in one 2-bank PSUM tile [128,1024] (den row spans both
    heads, one DVE copy per unit);
  * proj outputs accumulate per tq-group into resident [128,4096] bf16
    tiles, written back with one DMA per group (g=3 split in halves), so
    the tail is a short dense burst instead of 8 DMA-gated proj groups.

On-chip dataflow (per head pair, row/col layouts chosen so the TensorE
contraction dim is always the partition dim and no transposes are needed):
  qT,kT [d, t] -> S^T[tk, tq] (two heads packed in the 128-wide array via
  row tiling) -> exp on ScalarE (scale=1/8 folded in) -> causal mask via
  static 0/1 mask multiply on DVE -> AV matmul with V augmented by a ones
  column (denominator accumulates in row 64 of PSUM for free) -> bcast +
  reciprocal -> normalize -> projection (stays transposed).
"""

import os
import sys
import types

import numpy as np
import ml_dtypes

import concourse.bass as bass
import concourse.mybir as mybir
import concourse.tile as tile
from concourse import bacc
from concourse.hw_specs import get_activation_tables

BF16 = ml_dtypes.bfloat16


class _Bacc(bacc.Bacc):
    """Bacc that steers Exp/Ln activations to the combined
    natural_log_exp_and_others table set so the kernel never swaps
    activation tables (set ids keep their act_info.json positions)."""

    def insert_act_table_loads(self):
        import bass_rust as _br
        import concourse.mybir as _mybir

        has_activation = any(
            isinstance(i, _mybir.InstActivation)
            for b in self.main_func.blocks
            for i in b.instructions
        )
        if not has_activation:
            return
        combined = {"natural_log_exp_and_others"}
        steer = {_mybir.ActivationFunctionType.Exp, _mybir.ActivationFunctionType.Ln}
        tables = []
        for name, fns in get_activation_tables(self.m.arch).items():
            if name not in combined:
                fns = {f for f in fns if f not in steer}
            tables.append((name, set(fns)))
        _br.insert_act_table_loads(self, tables)

B, T, C = 2, 2048, 1024
H = 16
DH = 64
N_CORES = 8
HEADS_PER_CORE = 4
TQ = 512          # tq tile (moving dim of scores/AV matmuls)
TK = 128          # tk tile (PSUM partition dim of S^T)
NG = T // TQ      # 4 tq tiles
NKT = T // TK     # 16 tk tiles
NC_ = C // 128    # 8 contraction tiles for the qkv matmuls
FP32 = mybir.dt.float32
BF16_DT = mybir.dt.bfloat16
import os as _os
DEPTH = int(_os.environ.get("K_DEPTH", "4"))
POPS_EVERY = int(_os.environ.get("K_POPS_EVERY", "2"))
BOUNDARY_POPS = int(_os.environ.get("K_BPOPS", "2"))
WARMUP_MMS = int(_os.environ.get("K_WARMUP", "12"))
PT_BUFS = int(_os.environ.get("K_PT_BUFS", "6"))


def _ensure_axon_hooks_stub():
    """bass_utils imports antenv.axon_hooks when trace is requested (even via
    the BASS_TRACE env var). The container's antenv stub lacks that module, so
    install a minimal one to keep the no-trace fallback path working."""
    try:
        import antenv  # noqa: F401
    except ImportError:
        return
    if "antenv.axon_hooks" in sys.modules:
        return
    try:
        import antenv.axon_hooks  # noqa: F401
        return
    except ImportError:
        pass
    mod = types.ModuleType("antenv.axon_hooks")
    mod._hook = None

    def set_axon_ntff_profile_hook(h):
        mod._hook = h

    def get_axon_ntff_profile_hook():
        return mod._hook

    mod.set_axon_ntff_profile_hook = set_axon_ntff_profile_hook
    mod.get_axon_ntff_profile_hook = get_axon_ntff_profile_hook
    sys.modules["antenv.axon_hooks"] = mod
    import antenv as _a

    _a.axon_hooks = mod


def build_bass():
    """Emit the single-core SPMD Bass module (same program on all 8 cores)."""
    from collections import deque
    from contextlib import ExitStack

    nc = _Bacc("TRN2", target_bir_lowering=False, debug=False)

    xt = nc.declare_dram_parameter("xt", [C, T], BF16_DT, isOutput=False).ap()
    wqkt = nc.declare_dram_parameter("wqkt", [C, 512], BF16_DT, isOutput=False).ap()
    wvt = nc.declare_dram_parameter("wvt", [C, 256], BF16_DT, isOutput=False).ap()
    wpt = nc.declare_dram_parameter("wpt", [256, C], BF16_DT, isOutput=False).ap()
    bias = nc.declare_dram_parameter("bias", [128, 268], FP32, isOutput=False).ap()
    out_t = nc.declare_dram_parameter("out_t", [C, T], BF16_DT, isOutput=True).ap()

    Exp = mybir.ActivationFunctionType.Exp
    mult = mybir.AluOpType.mult
    add = mybir.AluOpType.add
    is_ge = mybir.AluOpType.is_ge

    with tile.TileContext(nc) as tc, ExitStack() as ctx:
        res = ctx.enter_context(tc.tile_pool(name="resident", bufs=1))

        # --- resident tiles (one big tile per input, sliced into views) ----
        xt_big = res.tile([128, NC_ * T], BF16_DT, tag="xt", name="xt_big")
        xt_t = [xt_big[:, T * i : T * (i + 1)] for i in range(NC_)]
        wqk_big = res.tile([128, NC_ * 512], BF16_DT, tag="wqk", name="wqk_big")
        wqk_t = [wqk_big[:, 512 * i : 512 * (i + 1)] for i in range(NC_)]
        wv_big = res.tile([128, NC_ * 256], BF16_DT, tag="wv", name="wv_big")
        wv_t = [wv_big[:, 256 * i : 256 * (i + 1)] for i in range(NC_)]
        wp_big = res.tile([128, 2 * C], BF16_DT, tag="wp", name="wp_big")
        wp_t = [wp_big[:, C * i : C * (i + 1)] for i in range(2)]
        bias_t = res.tile([128, 268], FP32, tag="bias", name="bias_t")
        bqk_t = [bias_t[:, j : j + 1] for j in range(4)]
        bp_t = [bias_t[:, 4 + j : 5 + j] for j in range(8)]
        bv_t = bias_t[:, 12:268]
        osb_t = [res.tile([128, NC_ * TQ], BF16_DT, tag=f"osb{g}", name=f"osb{g}")
                 for g in range(NG)]

        # --- PE warm-up first: dense zero matmuls while the DMAs stream in,
        # so the HAM clock gate opens before real compute starts ------------
        warm_sb = res.tile([128, 512], BF16_DT, tag="warm", name="warm_sb")
        nc.vector.memset(warm_sb[:], 0.0)

        sc_ps = ctx.enter_context(tc.tile_pool(name="sc_ps", bufs=2, space="PSUM"))
        av_ps = ctx.enter_context(tc.tile_pool(name="av_ps", bufs=1, space="PSUM"))
        qv_ps = ctx.enter_context(tc.tile_pool(name="qv_ps", bufs=1, space="PSUM"))
        bp_ps = ctx.enter_context(tc.tile_pool(name="bp_ps", bufs=1, space="PSUM"))
        pt_pool = ctx.enter_context(tc.tile_pool(name="pt_pool", bufs=PT_BUFS))
        riv_pool = ctx.enter_context(tc.tile_pool(name="riv", bufs=2))
        bcs_pool = ctx.enter_context(tc.tile_pool(name="bcs", bufs=2))
        scr_pool = ctx.enter_context(tc.tile_pool(name="scr", bufs=2))

        warm_ps = qv_ps.tile([128, 512], FP32, tag="qv", name="warm_ps")
        for i in range(WARMUP_MMS):
            nc.tensor.matmul(
                warm_ps[:], lhsT=warm_sb[:, 0:128], rhs=warm_sb[:],
                start=(i == 0), stop=(i == WARMUP_MMS - 1), skip_group_check=True,
            )

        # --- input loads: few large multi-dim-AP transfers ------------------
        # sync (HWDGE ring 1): xt in four 1MB t-blocks, first-needed first.
        xt_r = xt.rearrange("(i p) t -> p i t", p=128)
        xt_v = xt_big[:].rearrange("p (i t) -> p i t", t=T)
        for blk in range(NG):
            ts = slice(TQ * blk, TQ * (blk + 1))
            nc.sync.dma_start(xt_v[:, :, ts], xt_r[:, :, ts])
        # scalar (HWDGE ring 2, after its act-table load): weights.
        nc.scalar.dma_start(
            wqk_big[:].rearrange("p (i c) -> p i c", c=512),
            wqkt.rearrange("(i p) c -> p i c", p=128),
        )
        nc.scalar.dma_start(
            wv_big[:].rearrange("p (i c) -> p i c", c=256),
            wvt.rearrange("(i p) c -> p i c", p=128),
        )
        # gpsimd (SWDGE): biases (needed by the first qk group) and wp.
        nc.gpsimd.dma_start(bias_t[:], bias[:])
        nc.gpsimd.dma_start(
            wp_big[:].rearrange("p (i c) -> p i c", c=C),
            wpt.rearrange("(i p) c -> p i c", p=128),
        )

        # Single causal strip mask [128, 128]: keep iff local tq >= local tk.
        maskd = res.tile([128, 128], BF16_DT, tag="maskd", name="maskd")
        nc.gpsimd.memset(maskd[:], 1.0)
        nc.gpsimd.affine_select(
            out=maskd[:], in_=maskd[:], compare_op=is_ge, fill=0.0,
            base=0, pattern=[[1, 128]], channel_multiplier=-1,
        )

        # Ones row (lane 64, matching the av denominator row) for the K=1
        # broadcast matmuls.
        ones_t = res.tile([65, 64], BF16_DT, tag="ones_t", name="ones_t")
        nc.vector.memset(ones_t[:], 1.0)

        # qT/kT in [head-channel, t] layout: tile p holds heads (2p, 2p+1).
        qk_sb = [
            res.tile([128, T], BF16_DT, tag=f"qk{i}", name=f"qk{i}") for i in range(4)
        ]
        # V natural [t, d] with a ones column after each head: 4*(64+1) cols.
        v_sb = []
        for i in range(NKT):
            t = res.tile([128, 260], BF16_DT, tag=f"v{i}", name=f"v{i}")
            nc.gpsimd.memset(
                t[:].rearrange("p (h c) -> p h c", c=65)[:, :, 64:65], 1.0
            )
            v_sb.append(t)
        att_sb = [
            res.tile([128, T], BF16_DT, tag=f"att{i}", name=f"att{i}")
            for i in range(2)
        ]

        out_r = out_t.rearrange("(i p) t -> p i t", p=128)

        # --- filler work: qkv projections + output projection --------------
        emitted = set()

        def emit_qk_group(jt, g, pool=None):
            pool = pool or qv_ps
            ps = pool.tile([128, 512], FP32, tag="bp" if pool is bp_ps else "qv",
                           name=f"qkps{jt}_{g}")
            for ct in range(NC_):
                nc.tensor.matmul(
                    ps[:],
                    lhsT=wqk_t[ct][:, 128 * jt : 128 * (jt + 1)],
                    rhs=xt_t[ct][:, TQ * g : TQ * (g + 1)],
                    start=(ct == 0),
                    stop=(ct == NC_ - 1),
                )

            nc.vector.tensor_scalar(
                qk_sb[jt][:, TQ * g : TQ * (g + 1)], ps[:], bqk_t[jt][:], None,
                op0=add,
            )

        def emit_v_group(tt, pool=None):
            pool = pool or qv_ps
            ps = pool.tile([128, 512], FP32, tag="bp" if pool is bp_ps else "qv",
                           name=f"vps{tt}")
            for ct in range(NC_):
                nc.tensor.matmul(
                    ps[:, 0:256],
                    lhsT=xt_t[ct][:, 128 * tt : 128 * (tt + 1)],
                    rhs=wv_t[ct][:],
                    start=(ct == 0),
                    stop=(ct == NC_ - 1),
                )

            vt = v_sb[tt]
            nc.vector.tensor_tensor(
                out=vt[:].rearrange("p (h c) -> p h c", c=65)[:, :, 0:64],
                in0=ps[:, 0:256].rearrange("p (h c) -> p h c", c=64),
                in1=bv_t.rearrange("p (h c) -> p h c", c=64),
                op=add,
            )

        def emit_proj_group(jt, g, pool=None, tag="bp"):
            tqs = slice(TQ * g, TQ * (g + 1))
            pp = (pool or bp_ps).tile([128, 512], FP32, tag=tag, name=f"pj{g}{jt}")
            nc.tensor.matmul(
                pp[:], lhsT=wp_t[0][:, 128 * jt : 128 * (jt + 1)],
                rhs=att_sb[0][:, tqs], start=True, stop=False,
            )
            nc.tensor.matmul(
                pp[:], lhsT=wp_t[1][:, 128 * jt : 128 * (jt + 1)],
                rhs=att_sb[1][:, tqs], start=False, stop=True,
            )
            nc.vector.tensor_scalar(
                osb_t[g][:, TQ * jt : TQ * (jt + 1)], pp[:], bp_t[jt][:], None,
                op0=add,
            )
            # Stream the finished tq-group back: one DMA per g (g=3 halved so
            # the first half overlaps the second half's proj matmuls).
            osb_v = osb_t[g][:].rearrange("p (i t) -> p i t", t=TQ)
            if g < 3:
                if jt == 7:
                    nc.sync.dma_start(out_r[:, :, tqs], osb_v)
            else:
                if jt == 3:
                    nc.sync.dma_start(out_r[:, 0:4, tqs], osb_v[:, 0:4, :])
                elif jt == 7:
                    nc.sync.dma_start(out_r[:, 4:8, tqs], osb_v[:, 4:8, :])

        work_q = deque()

        # Dummy zero-matmuls to keep the PE clock gate open when real filler
        # runs dry (late units).
        hb_n = [0]

        def heartbeat(n=2, pool=None, tag="qv"):
            t = (pool or qv_ps).tile([128, 512], FP32, tag=tag,
                                     name=f"hb{hb_n[0]}")
            hb_n[0] += 1
            for i in range(n):
                nc.tensor.matmul(
                    t[:], lhsT=warm_sb[:, 0:128], rhs=warm_sb[:],
                    start=(i == 0), stop=(i == n - 1), skip_group_check=True,
                )

        def emit_item(item, pool=None):
            if item[0] == "qk":
                emit_qk_group(item[1], item[2], pool=pool)
            elif item[0] == "v":
                emit_v_group(item[1], pool=pool)
            else:
                emit_proj_group(item[1], item[2])
            emitted.add(item)

        def pop_one(force=False):
            if work_q:
                emit_item(work_q.popleft())

        def drain_until(needed):
            for item in needed:
                while item not in emitted:
                    emit_item(work_q.popleft())

        # prologue: enough qkv for unit (0, 0), rest queued in dep-safe order.
        # Alternate PSUM banks (qv/bp) so the DVE bias-add of one group
        # overlaps the matmuls of the next.
        for n, item in enumerate([("qk", 0, 0), ("qk", 2, 0), ("v", 0),
                                  ("v", 1), ("v", 2), ("v", 3)]):
            emit_item(item, pool=(bp_ps if n % 2 else qv_ps))
        work_q.extend([("qk", 1, 0), ("qk", 3, 0)])
        for gg in range(1, NG):
            work_q.extend(
                [("qk", 2, gg), ("qk", 0, gg), ("qk", 3, gg), ("qk", 1, gg)]
                + [("v", 4 * gg + i) for i in range(4)]
            )

        # --- attention: software-pipelined units -----------------------------
        def norm_pre(g, p, av_full):
            """Evacuate the packed den row [den_e | den_o] to bf16 SBUF."""
            den_b = riv_pool.tile([65, 1024], BF16_DT, tag="lr", name=f"dn{g}{p}")
            nc.vector.tensor_copy(out=den_b[64:65, :], in_=av_full[64:65, :])
            return den_b

        def norm_post(g, p, av_full, den_b):
            """Broadcast den across 64 partitions (K=1 matmul), invert with
            the fast DVE reciprocal, normalize; enqueues proj for p==1."""
            tqs = slice(TQ * g, TQ * (g + 1))
            bc_e = bp_ps.tile([64, 512], FP32, tag="bp", name=f"bce{g}{p}")
            nc.tensor.matmul(
                bc_e[:], lhsT=ones_t[64:65, :], rhs=den_b[64:65, 0:512],
                start=True, stop=True, tile_position=(64, 0),
            )
            bcs_e = bcs_pool.tile([64, 512], FP32, tag="bcs", name=f"bcse{g}{p}")
            nc.vector.reciprocal_approx_fast(out=bcs_e[:], in_=bc_e[:])
            nc.vector.tensor_tensor(
                out=att_sb[p][0:64, tqs], in0=av_full[0:64, 0:512], in1=bcs_e[:],
                op=mult,
            )
            bc_o = bp_ps.tile([64, 512], FP32, tag="bp", name=f"bco{g}{p}")
            nc.tensor.matmul(
                bc_o[:], lhsT=ones_t[64:65, :], rhs=den_b[64:65, 512:1024],
                start=True, stop=True, tile_position=(64, 0),
            )
            bcs_o = bcs_pool.tile([64, 512], FP32, tag="bcs", name=f"bcso{g}{p}")
            nc.vector.reciprocal_approx_fast(out=bcs_o[:], in_=bc_o[:])
            scr = scr_pool.tile([64, 512], BF16_DT, tag="scr", name=f"scr{g}{p}")
            nc.vector.tensor_tensor(
                out=scr[:], in0=av_full[0:64, 512:1024], in1=bcs_o[:], op=mult
            )
            nc.sync.dma_start(att_sb[p][64:128, tqs], scr[:])
            if p == 1:
                work_q.extend([("proj", jt, g) for jt in range(8)])

        pending_norm = None
        for g, p in [(0, 0), (0, 1), (1, 0), (1, 1), (2, 0), (2, 1),
                     (3, 0), (3, 1)]:
                nkt = 4 * (g + 1)
                h_e, h_o = 2 * p, 2 * p + 1
                q_t, k_t = qk_sb[p], qk_sb[2 + p]
                tq0 = TQ * g
                drain_until(
                    [("qk", p, g)]
                    + [("qk", 2 + p, gg) for gg in range(g + 1)]
                    + [("v", t) for t in range(nkt)]
                )
                s_tiles = {}
                p_tiles = {}
                av_full = None

                def lo_of(kt, g=g):
                    i = kt - 4 * g
                    return 128 * i if i > 0 else 0

                def scores(kt, g=g, q_t=q_t, k_t=k_t, tq0=tq0, p=p):
                    lo = lo_of(kt, g)
                    s_pair = sc_ps.tile([128, 1024], FP32, tag="sc",
                                        name=f"s{g}{p}{kt}")
                    kts = slice(128 * kt, 128 * (kt + 1))
                    rq = slice(tq0 + lo, tq0 + 512)
                    nc.tensor.matmul(
                        s_pair[:, lo:512], lhsT=k_t[0:64, kts], rhs=q_t[0:64, rq],
                        start=True, stop=True,
                    )
                    nc.tensor.matmul(
                        s_pair[:, 512 + lo : 1024], lhsT=k_t[64:128, kts],
                        rhs=q_t[64:128, rq], start=True, stop=True,
                        tile_position=(64, 0),
                    )
                    s_tiles[kt] = s_pair

                def expmask(kt, g=g, p=p):
                    lo = lo_of(kt, g)
                    s_pair = s_tiles.pop(kt)
                    p_pair = pt_pool.tile([128, 1024], BF16_DT, tag="pt",
                                          name=f"p{g}{p}{kt}")
                    s3 = s_pair[:].rearrange("p (h c) -> p h c", c=512)[:, :, lo:512]
                    p3 = p_pair[:].rearrange("p (h c) -> p h c", c=512)[:, :, lo:512]
                    nc.scalar.activation(p3, s3, Exp, scale=0.125)
                    if kt >= 4 * g:  # diagonal: mask the leading 128-wide strip
                        pm = p_pair[:].rearrange("p (h c) -> p h c", c=512)[
                            :, :, lo : lo + 128
                        ]
                        mk = maskd[:, None, 0:128].to_broadcast([128, 2, 128])
                        nc.gpsimd.tensor_tensor(out=pm, in0=pm, in1=mk, op=mult)
                    p_tiles[kt] = p_pair

                def av_mm(kt, g=g, p=p, nkt=nkt, h_e=h_e, h_o=h_o):
                    lo = lo_of(kt, g)
                    p_pair = p_tiles.pop(kt)
                    nc.tensor.matmul(
                        av_full[0:65, lo:512],
                        lhsT=v_sb[kt][:, 65 * h_e : 65 * h_e + 65],
                        rhs=p_pair[:, lo:512], start=(kt == 0),
                        stop=(kt == nkt - 1), skip_group_check=True,
                    )
                    nc.tensor.matmul(
                        av_full[0:65, 512 + lo : 1024],
                        lhsT=v_sb[kt][:, 65 * h_o : 65 * h_o + 65],
                        rhs=p_pair[:, 512 + lo : 1024], start=(kt == 0),
                        stop=(kt == nkt - 1), skip_group_check=True,
                    )

                depth = min(DEPTH, nkt)
                for kt in range(2):
                    scores(kt)
                for kt in range(2):
                    expmask(kt)
                for kt in range(2, depth):
                    scores(kt)
                    expmask(kt)
                if pending_norm is not None:
                    den_b = norm_pre(*pending_norm)
                    for _ in range(BOUNDARY_POPS):
                        pop_one()
                    norm_post(*pending_norm, den_b)
                av_full = av_ps.tile([128, 1024], FP32, tag="av",
                                     name=f"av{g}{p}")
                for kt in range(depth, nkt):
                    scores(kt)
                    expmask(kt)
                    if kt % POPS_EVERY == 0:
                        if work_q:
                            pop_one()
                        elif g >= 2:
                            heartbeat(3)
                    av_mm(kt - depth)
                for kt in range(nkt - depth, nkt):
                    av_mm(kt)
                pending_norm = (g, p, av_full)

        # epilogue: last unit's normalize + the g=3 proj groups (these rotate
        # through the now-idle 2-bank score slots for more overlap)
        den_b = norm_pre(*pending_norm)
        norm_post(*pending_norm, den_b)
        while work_q:
            item = work_q.popleft()
            if item[0] == "proj":
                emit_proj_group(item[1], item[2], pool=sc_ps, tag="sc")
                emitted.add(item)
            else:
                emit_item(item)

    nc.compile()
    return nc


_NC_CACHE = None


def _get_nc():
    global _NC_CACHE
    if _NC_CACHE is None:
        _NC_CACHE = build_bass()
    return _NC_CACHE


def make_in_maps(x, w_attn, b_attn, w_proj, b_proj):
    """Host-side sharding: slice/transpose/cast the full inputs per core."""
    x = np.asarray(x, dtype=np.float32)
    w_attn = np.asarray(w_attn, dtype=np.float32)
    b_attn = np.asarray(b_attn, dtype=np.float32)
    w_proj = np.asarray(w_proj, dtype=np.float32)
    b_proj = np.asarray(b_proj, dtype=np.float32)
    in_maps = []
    for core in range(N_CORES):
        b = core // 4
        heads = [4 * (core % 4) + i for i in range(HEADS_PER_CORE)]
        ch = np.concatenate([np.arange(h * DH, (h + 1) * DH) for h in heads])
        idx_qk = np.concatenate([ch, C + ch])
        idx_v = 2 * C + ch
        bias_all = np.empty((128, 268), dtype=np.float32)
        bias_all[:, 0:4] = b_attn[idx_qk].reshape(4, 128).T
        bias_all[:, 4:12] = (b_proj / 4.0).reshape(8, 128).T
        bias_all[:, 12:268] = np.tile(b_attn[idx_v][None, :], (128, 1))
        in_maps.append(
            {
                "xt": np.ascontiguousarray(x[b].T).astype(BF16),
                "wqkt": np.ascontiguousarray(w_attn[idx_qk].T).astype(BF16),
                "wvt": np.ascontiguousarray(w_attn[idx_v].T).astype(BF16),
                "wpt": np.ascontiguousarray(w_proj[:, ch].T).astype(BF16),
                "bias": bias_all,
            }
        )
    return in_maps


def assemble_output(results):
    out = np.zeros((B, T, C), dtype=np.float32)
    for core in range(N_CORES):
        out[core // 4] += np.asarray(results[core]["out_t"], dtype=np.float32).T
    return out


def run(inputs, trace=False, trace_cores=None, tmpdir=None):
    """Run on hardware; returns (output, BassKernelResults)."""
    _ensure_axon_hooks_stub()
    from concourse.bass_utils import run_bass_kernel_spmd

    nc = _get_nc()
    in_maps = make_in_maps(**inputs)
    kw = {}
    if trace:
        kw.update(trace=True, trace_cores=trace_cores, tmpdir=tmpdir)
    res = run_bass_kernel_spmd(nc, in_maps, core_ids=list(range(N_CORES)), **kw)
    return assemble_output(res.results), res


def kernel(x, w_attn, b_attn, w_proj, b_proj):
    out, _ = run(
        dict(x=x, w_attn=w_attn, b_attn=b_attn, w_proj=w_proj, b_proj=b_proj)
    )
    return out


# revision 12
# speedup vs baseline: 1.1762x; 1.0468x over previous
"""Causal self-attention (B=2, T=2048, C=1024, H=16) on 8 TRN2 NeuronCores.

Sharding: core c -> batch b = c // 4, head group = heads [4*(c%4) .. 4*(c%4)+4).
Each core computes qkv for its 4 heads on its batch, causal attention, and a
row-parallel partial of the output projection (over its 256 head channels).
The host sums the 4 partials per batch; b_proj/4 is added on-device so the sum
reproduces a single b_proj add.  Output partials stream back in bf16.

All device tensors are pre-transposed on the host so the kernel never
transposes on-chip:
  xt   [C, T]    = x[b].T                     (bf16)
  wqkt [C, 512]  = w_attn[qk rows].T          (bf16)  cols: q_h0 q_h1 q_h2 q_h3 k_h0..k_h3
  wvt  [C, 256]  = w_attn[v rows].T           (bf16)
  wpt  [256, C]  = w_proj[:, head cols].T     (bf16)
  bias [128,268] = packed bqk(4) | bp(8) | bv(256) columns (fp32)
  out_t[C, T]    = partial (x @ w_proj.T).T   (bf16)

Round 3 (vs the round-2 baseline at ~194us):
  * input DMAs coalesced into 7 big multi-dim-AP transfers (xt in four
    1MB t-blocks on sync, wqk/wv on scalar, bias/wp on gpsimd) so the
    first qk group can start ~6us in instead of ~22us;
  * softmax 1/den moved off ScalarE: DVE copies the PSUM den row to bf16,
    a K=1 matmul broadcasts den across 64 partitions, and
    reciprocal_approx_fast (custom DVE op, ~18 bits) inverts it -- the
    Ln/Exp activation chain is gone (ScalarE now runs score exps only);
  * av pair kept in one 2-bank PSUM tile [128,1024] (den row spans both
    heads, one DVE copy per unit);
  * proj outputs accumulate per tq-group into resident [128,4096] bf16
    tiles, written back with one DMA per group (g=3 split in halves), so
    the tail is a short dense burst instead of 8 DMA-gated proj groups.

On-chip dataflow (per head pair, row/col layouts chosen so the TensorE
contraction dim is always the partition dim and no transposes are needed):
  qT,kT [d, t] -> S^T[tk, tq] (two heads packed in the 128-wide array via
  row tiling) -> exp on ScalarE (scale=1/8 folded in) -> causal mask via
  static 0/1 mask multiply on DVE -> AV matmul with V augmented by a ones
  column (denominator accumulates in row 64 of PSUM for free) -> bcast +
  reciprocal -> normalize -> projection (stays transposed).
"""

import os
import sys
import types

import numpy as np
import ml_dtypes

import concourse.bass as bass
import concourse.mybir as mybir
import concourse.tile as tile
from concourse import bacc
from concourse.hw_specs import get_activation_tables

BF16 = ml_dtypes.bfloat16


class _Bacc(bacc.Bacc):
    """Bacc that steers Exp/Ln activations to the combined
    natural_log_exp_and_others table set so the kernel never swaps
    activation tables (set ids keep their act_info.json positions)."""

    def insert_act_table_loads(self):
        import bass_rust as _br
        import concourse.mybir as _mybir

        has_activation = any(
            isinstance(i, _mybir.InstActivation)
            for b in self.main_func.blocks
            for i in b.instructions
        )
        if not has_activation:
            return
        combined = {"natural_log_exp_and_others"}
        steer = {_mybir.ActivationFunctionType.Exp, _mybir.ActivationFunctionType.Ln}
        tables = []
        for name, fns in get_activation_tables(self.m.arch).items():
            if name not in combined:
                fns = {f for f in fns if f not in steer}
            tables.append((name, set(fns)))
        _br.insert_act_table_loads(self, tables)

B, T, C = 2, 2048, 1024
H = 16
DH = 64
N_CORES = 8
HEADS_PER_CORE = 4
TQ = 512          # tq tile (moving dim of scores/AV matmuls)
TK = 128          # tk tile (PSUM partition dim of S^T)
NG = T // TQ      # 4 tq tiles
NKT = T // TK     # 16 tk tiles
NC_ = C // 128    # 8 contraction tiles for the qkv matmuls
FP32 = mybir.dt.float32
BF16_DT = mybir.dt.bfloat16
import os as _os
DEPTH = int(_os.environ.get("K_DEPTH", "4"))
POPS_EVERY = int(_os.environ.get("K_POPS_EVERY", "2"))
BOUNDARY_POPS = int(_os.environ.get("K_BPOPS", "2"))
WARMUP_MMS = int(_os.environ.get("K_WARMUP", "12"))
PT_BUFS = int(_os.environ.get("K_PT_BUFS", "6"))


def _ensure_axon_hooks_stub():
    """bass_utils imports antenv.axon_hooks when trace is requested (even via
    the BASS_TRACE env var). The container's antenv stub lacks that module, so
    install a minimal one to keep the no-trace fallback path working."""
    try:
        import antenv  # noqa: F401
    except ImportError:
        return
    if "antenv.axon_hooks" in sys.modules:
        return
    try:
        import antenv.axon_hooks  # noqa: F401
        return
    except ImportError:
        pass
    mod = types.ModuleType("antenv.axon_hooks")
    mod._hook = None

    def set_axon_ntff_profile_hook(h):
        mod._hook = h

    def get_axon_ntff_profile_hook():
        return mod._hook

    mod.set_axon_ntff_profile_hook = set_axon_ntff_profile_hook
    mod.get_axon_ntff_profile_hook = get_axon_ntff_profile_hook
    sys.modules["antenv.axon_hooks"] = mod
    import antenv as _a

    _a.axon_hooks = mod


def build_bass():
    """Emit the single-core SPMD Bass module (same program on all 8 cores)."""
    from collections import deque
    from contextlib import ExitStack

    nc = _Bacc("TRN2", target_bir_lowering=False, debug=False)

    xt = nc.declare_dram_parameter("xt", [C, T], BF16_DT, isOutput=False).ap()
    wqkt = nc.declare_dram_parameter("wqkt", [C, 512], BF16_DT, isOutput=False).ap()
    wvt = nc.declare_dram_parameter("wvt", [C, 256], BF16_DT, isOutput=False).ap()
    wpt = nc.declare_dram_parameter("wpt", [256, C], BF16_DT, isOutput=False).ap()
    bias = nc.declare_dram_parameter("bias", [128, 268], FP32, isOutput=False).ap()
    out_t = nc.declare_dram_parameter("out_t", [C, T], BF16_DT, isOutput=True).ap()

    Exp = mybir.ActivationFunctionType.Exp
    mult = mybir.AluOpType.mult
    add = mybir.AluOpType.add
    is_ge = mybir.AluOpType.is_ge

    with tile.TileContext(nc) as tc, ExitStack() as ctx:
        res = ctx.enter_context(tc.tile_pool(name="resident", bufs=1))

        # --- resident tiles (one big tile per input, sliced into views) ----
        xt_big = res.tile([128, NC_ * T], BF16_DT, tag="xt", name="xt_big")
        xt_t = [xt_big[:, T * i : T * (i + 1)] for i in range(NC_)]
        wqk_big = res.tile([128, NC_ * 512], BF16_DT, tag="wqk", name="wqk_big")
        wqk_t = [wqk_big[:, 512 * i : 512 * (i + 1)] for i in range(NC_)]
        wv_big = res.tile([128, NC_ * 256], BF16_DT, tag="wv", name="wv_big")
        wv_t = [wv_big[:, 256 * i : 256 * (i + 1)] for i in range(NC_)]
        wp_big = res.tile([128, 2 * C], BF16_DT, tag="wp", name="wp_big")
        wp_t = [wp_big[:, C * i : C * (i + 1)] for i in range(2)]
        bias_t = res.tile([128, 268], FP32, tag="bias", name="bias_t")
        bqk_t = [bias_t[:, j : j + 1] for j in range(4)]
        bp_t = [bias_t[:, 4 + j : 5 + j] for j in range(8)]
        bv_t = bias_t[:, 12:268]
        osb_t = [res.tile([128, NC_ * TQ], BF16_DT, tag=f"osb{g}", name=f"osb{g}")
                 for g in range(NG)]

        # --- PE warm-up first: dense zero matmuls while the DMAs stream in,
        # so the HAM clock gate opens before real compute starts ------------
        warm_sb = res.tile([128, 512], BF16_DT, tag="warm", name="warm_sb")
        nc.vector.memset(warm_sb[:], 0.0)

        sc_ps = ctx.enter_context(tc.tile_pool(name="sc_ps", bufs=2, space="PSUM"))
        av_ps = ctx.enter_context(tc.tile_pool(name="av_ps", bufs=1, space="PSUM"))
        qv_ps = ctx.enter_context(tc.tile_pool(name="qv_ps", bufs=1, space="PSUM"))
        bp_ps = ctx.enter_context(tc.tile_pool(name="bp_ps", bufs=1, space="PSUM"))
        pt_pool = ctx.enter_context(tc.tile_pool(name="pt_pool", bufs=PT_BUFS))
        riv_pool = ctx.enter_context(tc.tile_pool(name="riv", bufs=2))
        bcs_pool = ctx.enter_context(tc.tile_pool(name="bcs", bufs=2))
        scr_pool = ctx.enter_context(tc.tile_pool(name="scr", bufs=2))

        warm_ps = qv_ps.tile([128, 512], FP32, tag="qv", name="warm_ps")
        for i in range(WARMUP_MMS):
            nc.tensor.matmul(
                warm_ps[:], lhsT=warm_sb[:, 0:128], rhs=warm_sb[:],
                start=(i == 0), stop=(i == WARMUP_MMS - 1), skip_group_check=True,
            )

        # --- input loads: few large multi-dim-AP transfers ------------------
        # sync (HWDGE ring 1): xt in four 1MB t-blocks, first-needed first.
        xt_r = xt.rearrange("(i p) t -> p i t", p=128)
        xt_v = xt_big[:].rearrange("p (i t) -> p i t", t=T)
        for blk in range(NG):
            ts = slice(TQ * blk, TQ * (blk + 1))
            nc.sync.dma_start(xt_v[:, :, ts], xt_r[:, :, ts])
        # scalar (HWDGE ring 2, after its act-table load): weights, in
        # priority order -- queue FIFO keeps wv/wp from competing with the
        # critical wqk for HBM bandwidth.
        nc.scalar.dma_start(
            wqk_big[:].rearrange("p (i c) -> p i c", c=512),
            wqkt.rearrange("(i p) c -> p i c", p=128),
        )
        nc.scalar.dma_start(
            wv_big[:].rearrange("p (i c) -> p i c", c=256),
            wvt.rearrange("(i p) c -> p i c", p=128),
        )
        nc.scalar.dma_start(
            wp_big[:].rearrange("p (i c) -> p i c", c=C),
            wpt.rearrange("(i p) c -> p i c", p=128),
        )
        # gpsimd (SWDGE): just the tiny bias pack (needed by the first qk
        # group's bias add, lands ~2us).
        nc.gpsimd.dma_start(bias_t[:], bias[:])

        # Single causal strip mask [128, 128]: keep iff local tq >= local tk.
        maskd = res.tile([128, 128], BF16_DT, tag="maskd", name="maskd")
        nc.gpsimd.memset(maskd[:], 1.0)
        nc.gpsimd.affine_select(
            out=maskd[:], in_=maskd[:], compare_op=is_ge, fill=0.0,
            base=0, pattern=[[1, 128]], channel_multiplier=-1,
        )

        # Ones row (lane 64, matching the av denominator row) for the K=1
        # broadcast matmuls.
        ones_t = res.tile([65, 64], BF16_DT, tag="ones_t", name="ones_t")
        nc.vector.memset(ones_t[:], 1.0)

        # qT/kT in [head-channel, t] layout: tile p holds heads (2p, 2p+1).
        qk_sb = [
            res.tile([128, T], BF16_DT, tag=f"qk{i}", name=f"qk{i}") for i in range(4)
        ]
        # V natural [t, d] with a ones column after each head: 4*(64+1) cols.
        v_sb = []
        for i in range(NKT):
            t = res.tile([128, 260], BF16_DT, tag=f"v{i}", name=f"v{i}")
            nc.gpsimd.memset(
                t[:].rearrange("p (h c) -> p h c", c=65)[:, :, 64:65], 1.0
            )
            v_sb.append(t)
        att_sb = [
            res.tile([128, T], BF16_DT, tag=f"att{i}", name=f"att{i}")
            for i in range(2)
        ]

        out_r = out_t.rearrange("(i p) t -> p i t", p=128)

        # --- filler work: qkv projections + output projection --------------
        emitted = set()

        def emit_qk_group(jt, g, pool=None):
            pool = pool or qv_ps
            ps = pool.tile([128, 512], FP32, tag="bp" if pool is bp_ps else "qv",
                           name=f"qkps{jt}_{g}")
            for ct in range(NC_):
                nc.tensor.matmul(
                    ps[:],
                    lhsT=wqk_t[ct][:, 128 * jt : 128 * (jt + 1)],
                    rhs=xt_t[ct][:, TQ * g : TQ * (g + 1)],
                    start=(ct == 0),
                    stop=(ct == NC_ - 1),
                )

            nc.vector.tensor_scalar(
                qk_sb[jt][:, TQ * g : TQ * (g + 1)], ps[:], bqk_t[jt][:], None,
                op0=add,
            )

        def emit_v_group(tt, pool=None):
            pool = pool or qv_ps
            ps = pool.tile([128, 512], FP32, tag="bp" if pool is bp_ps else "qv",
                           name=f"vps{tt}")
            for ct in range(NC_):
                nc.tensor.matmul(
                    ps[:, 0:256],
                    lhsT=xt_t[ct][:, 128 * tt : 128 * (tt + 1)],
                    rhs=wv_t[ct][:],
                    start=(ct == 0),
                    stop=(ct == NC_ - 1),
                )

            vt = v_sb[tt]
            nc.vector.tensor_tensor(
                out=vt[:].rearrange("p (h c) -> p h c", c=65)[:, :, 0:64],
                in0=ps[:, 0:256].rearrange("p (h c) -> p h c", c=64),
                in1=bv_t.rearrange("p (h c) -> p h c", c=64),
                op=add,
            )

        def emit_proj_group(jt, g, pool=None, tag="bp"):
            tqs = slice(TQ * g, TQ * (g + 1))
            pp = (pool or bp_ps).tile([128, 512], FP32, tag=tag, name=f"pj{g}{jt}")
            nc.tensor.matmul(
                pp[:], lhsT=wp_t[0][:, 128 * jt : 128 * (jt + 1)],
                rhs=att_sb[0][:, tqs], start=True, stop=False,
            )
            nc.tensor.matmul(
                pp[:], lhsT=wp_t[1][:, 128 * jt : 128 * (jt + 1)],
                rhs=att_sb[1][:, tqs], start=False, stop=True,
            )
            nc.vector.tensor_scalar(
                osb_t[g][:, TQ * jt : TQ * (jt + 1)], pp[:], bp_t[jt][:], None,
                op0=add,
            )
            # Stream the finished tq-group back: one DMA per g (g=3 quartered
            # across both HWDGE rings so the tail transfers overlap the
            # remaining proj matmuls).
            osb_v = osb_t[g][:].rearrange("p (i t) -> p i t", t=TQ)
            if g < 3:
                if jt == 7:
                    nc.sync.dma_start(out_r[:, :, tqs], osb_v)
            elif jt % 2 == 1:
                q = slice(jt - 1, jt + 1)
                eng = nc.sync if jt % 4 == 1 else nc.scalar
                eng.dma_start(out_r[:, q, tqs], osb_v[:, q, :])

        work_q = deque()

        # Dummy zero-matmuls to keep the PE clock gate open when real filler
        # runs dry (late units).
        hb_n = [0]

        def heartbeat(n=2, pool=None, tag="qv"):
            t = (pool or qv_ps).tile([128, 512], FP32, tag=tag,
                                     name=f"hb{hb_n[0]}")
            hb_n[0] += 1
            for i in range(n):
                nc.tensor.matmul(
                    t[:], lhsT=warm_sb[:, 0:128], rhs=warm_sb[:],
                    start=(i == 0), stop=(i == n - 1), skip_group_check=True,
                )

        def emit_item(item, pool=None):
            if item[0] == "qk":
                emit_qk_group(item[1], item[2], pool=pool)
            elif item[0] == "v":
                emit_v_group(item[1], pool=pool)
            else:
                emit_proj_group(item[1], item[2])
            emitted.add(item)

        def pop_one(force=False):
            if work_q:
                emit_item(work_q.popleft())

        def drain_until(needed):
            for item in needed:
                while item not in emitted:
                    emit_item(work_q.popleft())

        # prologue: enough qkv for unit (0, 0), rest queued in dep-safe order.
        # Alternate PSUM banks (qv/bp) so the DVE bias-add of one group
        # overlaps the matmuls of the next.
        for n, item in enumerate([("qk", 0, 0), ("qk", 2, 0), ("v", 0),
                                  ("v", 1), ("v", 2), ("v", 3)]):
            emit_item(item, pool=(bp_ps if n % 2 else qv_ps))
        work_q.extend([("qk", 1, 0), ("qk", 3, 0)])
        for gg in range(1, NG):
            work_q.extend(
                [("qk", 2, gg), ("qk", 0, gg), ("qk", 3, gg), ("qk", 1, gg)]
                + [("v", 4 * gg + i) for i in range(4)]
            )

        # --- attention: software-pipelined units -----------------------------
        def norm_pre(g, p, av_full):
            """Evacuate the packed den row [den_e | den_o] to bf16 SBUF."""
            den_b = riv_pool.tile([65, 1024], BF16_DT, tag="lr", name=f"dn{g}{p}")
            nc.vector.tensor_copy(out=den_b[64:65, :], in_=av_full[64:65, :])
            return den_b

        def norm_post(g, p, av_full, den_b):
            """Broadcast den across 64 partitions (K=1 matmul), invert with
            the fast DVE reciprocal, normalize; enqueues proj for p==1.
            Odd head first so its cross-partition SBUF DMA overlaps the even
            head's normalize chain."""
            tqs = slice(TQ * g, TQ * (g + 1))
            last = (g == 3 and p == 1)
            bc_o = bp_ps.tile([64, 512], FP32, tag="bp", name=f"bco{g}{p}")
            nc.tensor.matmul(
                bc_o[:], lhsT=ones_t[64:65, :], rhs=den_b[64:65, 512:1024],
                start=True, stop=True, tile_position=(64, 0),
            )
            bcs_o = bcs_pool.tile([64, 512], FP32, tag="bcs", name=f"bcso{g}{p}")
            nc.vector.reciprocal_approx_fast(out=bcs_o[:], in_=bc_o[:])
            scr = scr_pool.tile([64, 512], BF16_DT, tag="scr", name=f"scr{g}{p}")
            nc.vector.tensor_tensor(
                out=scr[:], in0=av_full[0:64, 512:1024], in1=bcs_o[:], op=mult
            )
            (nc.scalar if last else nc.sync).dma_start(
                att_sb[p][64:128, tqs], scr[:]
            )
            bc_e = bp_ps.tile([64, 512], FP32, tag="bp", name=f"bce{g}{p}")
            nc.tensor.matmul(
                bc_e[:], lhsT=ones_t[64:65, :], rhs=den_b[64:65, 0:512],
                start=True, stop=True, tile_position=(64, 0),
            )
            bcs_e = bcs_pool.tile([64, 512], FP32, tag="bcs", name=f"bcse{g}{p}")
            nc.vector.reciprocal_approx_fast(out=bcs_e[:], in_=bc_e[:])
            nc.vector.tensor_tensor(
                out=att_sb[p][0:64, tqs], in0=av_full[0:64, 0:512], in1=bcs_e[:],
                op=mult,
            )
            if p == 1:
                work_q.extend([("proj", jt, g) for jt in range(8)])

        pending_norm = None
        carry = []
        for g, p in [(0, 0), (0, 1), (1, 0), (1, 1), (2, 0), (2, 1),
                     (3, 0), (3, 1)]:
                nkt = 4 * (g + 1)
                h_e, h_o = 2 * p, 2 * p + 1
                q_t, k_t = qk_sb[p], qk_sb[2 + p]
                tq0 = TQ * g
                drain_until(
                    [("qk", p, g)]
                    + [("qk", 2 + p, gg) for gg in range(g + 1)]
                    + [("v", t) for t in range(nkt)]
                )
                s_tiles = {}
                p_tiles = {}
                av_full = None

                def lo_of(kt, g=g):
                    i = kt - 4 * g
                    return 128 * i if i > 0 else 0

                def scores(kt, g=g, q_t=q_t, k_t=k_t, tq0=tq0, p=p):
                    lo = lo_of(kt, g)
                    s_pair = sc_ps.tile([128, 1024], FP32, tag="sc",
                                        name=f"s{g}{p}{kt}")
                    kts = slice(128 * kt, 128 * (kt + 1))
                    rq = slice(tq0 + lo, tq0 + 512)
                    nc.tensor.matmul(
                        s_pair[:, lo:512], lhsT=k_t[0:64, kts], rhs=q_t[0:64, rq],
                        start=True, stop=True,
                    )
                    nc.tensor.matmul(
                        s_pair[:, 512 + lo : 1024], lhsT=k_t[64:128, kts],
                        rhs=q_t[64:128, rq], start=True, stop=True,
                        tile_position=(64, 0),
                    )
                    s_tiles[kt] = s_pair

                def expmask(kt, g=g, p=p):
                    lo = lo_of(kt, g)
                    s_pair = s_tiles.pop(kt)
                    p_pair = pt_pool.tile([128, 1024], BF16_DT, tag="pt",
                                          name=f"p{g}{p}{kt}")
                    s3 = s_pair[:].rearrange("p (h c) -> p h c", c=512)[:, :, lo:512]
                    p3 = p_pair[:].rearrange("p (h c) -> p h c", c=512)[:, :, lo:512]
                    nc.scalar.activation(p3, s3, Exp, scale=0.125)
                    if kt >= 4 * g:  # diagonal: mask the leading 128-wide strip
                        pm = p_pair[:].rearrange("p (h c) -> p h c", c=512)[
                            :, :, lo : lo + 128
                        ]
                        mk = maskd[:, None, 0:128].to_broadcast([128, 2, 128])
                        nc.gpsimd.tensor_tensor(out=pm, in0=pm, in1=mk, op=mult)
                    p_tiles[kt] = p_pair

                def av_mm(kt, av, pt, g=g, nkt=nkt, h_e=h_e, h_o=h_o):
                    lo = lo_of(kt, g)
                    p_pair = pt.pop(kt)
                    nc.tensor.matmul(
                        av[0:65, lo:512],
                        lhsT=v_sb[kt][:, 65 * h_e : 65 * h_e + 65],
                        rhs=p_pair[:, lo:512], start=(kt == 0),
                        stop=(kt == nkt - 1), skip_group_check=True,
                    )
                    nc.tensor.matmul(
                        av[0:65, 512 + lo : 1024],
                        lhsT=v_sb[kt][:, 65 * h_o : 65 * h_o + 65],
                        rhs=p_pair[:, 512 + lo : 1024], start=(kt == 0),
                        stop=(kt == nkt - 1), skip_group_check=True,
                    )

                def drain_carry(n=1):
                    for _ in range(n):
                        if carry:
                            carry.pop(0)()

                # Last `depth` AV matmuls of the previous unit interleave
                # with this unit's score/exp prologue, so the previous exp
                # chain finishes while the PE stays on fresh scores.
                depth = min(DEPTH, nkt)
                for kt in range(2):
                    scores(kt)
                    drain_carry()
                for kt in range(2):
                    expmask(kt)
                    drain_carry()
                for kt in range(2, depth):
                    scores(kt)
                    expmask(kt)
                    drain_carry()
                drain_carry(len(carry))
                if pending_norm is not None:
                    den_b = norm_pre(*pending_norm)
                    for _ in range(BOUNDARY_POPS):
                        pop_one()
                    norm_post(*pending_norm, den_b)
                av_full = av_ps.tile([128, 1024], FP32, tag="av",
                                     name=f"av{g}{p}")
                for kt in range(depth, nkt):
                    scores(kt)
                    expmask(kt)
                    if kt % POPS_EVERY == 0:
                        if work_q:
                            pop_one()
                        elif g >= 2:
                            heartbeat(3)
                    av_mm(kt - depth, av_full, p_tiles)
                carry = [
                    (lambda kt=kt, av=av_full, pt=p_tiles, f=av_mm:
                     f(kt, av, pt))
                    for kt in range(nkt - depth, nkt)
                ]
                pending_norm = (g, p, av_full)

        # epilogue: drain the last unit's AV carry with heartbeats covering
        # the exp chain, then normalize and run the g=3 proj groups (these
        # rotate through the now-idle 2-bank score slots for more overlap)
        while carry:
            carry.pop(0)()
            heartbeat(2)
        den_b = norm_pre(*pending_norm)
        norm_post(*pending_norm, den_b)
        while work_q:
            item = work_q.popleft()
            if item[0] == "proj":
                emit_proj_group(item[1], item[2], pool=sc_ps, tag="sc")
                emitted.add(item)
            else:
                emit_item(item)

    nc.compile()
    return nc


_NC_CACHE = None


def _get_nc():
    global _NC_CACHE
    if _NC_CACHE is None:
        _NC_CACHE = build_bass()
    return _NC_CACHE


def make_in_maps(x, w_attn, b_attn, w_proj, b_proj):
    """Host-side sharding: slice/transpose/cast the full inputs per core."""
    x = np.asarray(x, dtype=np.float32)
    w_attn = np.asarray(w_attn, dtype=np.float32)
    b_attn = np.asarray(b_attn, dtype=np.float32)
    w_proj = np.asarray(w_proj, dtype=np.float32)
    b_proj = np.asarray(b_proj, dtype=np.float32)
    in_maps = []
    for core in range(N_CORES):
        b = core // 4
        heads = [4 * (core % 4) + i for i in range(HEADS_PER_CORE)]
        ch = np.concatenate([np.arange(h * DH, (h + 1) * DH) for h in heads])
        idx_qk = np.concatenate([ch, C + ch])
        idx_v = 2 * C + ch
        bias_all = np.empty((128, 268), dtype=np.float32)
        bias_all[:, 0:4] = b_attn[idx_qk].reshape(4, 128).T
        bias_all[:, 4:12] = (b_proj / 4.0).reshape(8, 128).T
        bias_all[:, 12:268] = np.tile(b_attn[idx_v][None, :], (128, 1))
        in_maps.append(
            {
                "xt": np.ascontiguousarray(x[b].T).astype(BF16),
                "wqkt": np.ascontiguousarray(w_attn[idx_qk].T).astype(BF16),
                "wvt": np.ascontiguousarray(w_attn[idx_v].T).astype(BF16),
                "wpt": np.ascontiguousarray(w_proj[:, ch].T).astype(BF16),
                "bias": bias_all,
            }
        )
    return in_maps


def assemble_output(results):
    out = np.zeros((B, T, C), dtype=np.float32)
    for core in range(N_CORES):
        out[core // 4] += np.asarray(results[core]["out_t"], dtype=np.float32).T
    return out


def run(inputs, trace=False, trace_cores=None, tmpdir=None):
    """Run on hardware; returns (output, BassKernelResults)."""
    _ensure_axon_hooks_stub()
    from concourse.bass_utils import run_bass_kernel_spmd

    nc = _get_nc()
    in_maps = make_in_maps(**inputs)
    kw = {}
    if trace:
        kw.update(trace=True, trace_cores=trace_cores, tmpdir=tmpdir)
    res = run_bass_kernel_spmd(nc, in_maps, core_ids=list(range(N_CORES)), **kw)
    return assemble_output(res.results), res


def kernel(x, w_attn, b_attn, w_proj, b_proj):
    out, _ = run(
        dict(x=x, w_attn=w_attn, b_attn=b_attn, w_proj=w_proj, b_proj=b_proj)
    )
    return out
